# revision 10
# baseline (speedup 1.0000x reference)
"""GRU sampler kernel for Trainium2 (8 NeuronCores, batch-data-parallel).

Reference computation (per batch row, T=64 steps):
    codes0 = sigmoid(noise @ W_out.T + b_out)
    per step: gi = codes @ W_ih.T + b_ih ; gh = h @ W_hh.T + b_hh
              r = sig(gi_r + gh_r); z = sig(gi_z + gh_z)
              n = tanh(gi_n + r * gh_n); h' = (1-z)*n + z*h
              codes' = sigmoid(h' @ W_out.T + b_out)
    samples[t] = codes (pre-cell), hiddens[t] = h' (post-cell)

Strategy: batch 256 split 8 ways (32/core). Weights cast to bf16 and kept
resident in SBUF (fp32 would not fit). Activations are the matmul
*stationary* operand (feature-on-partition, only 32 columns -> cheap
LDWEIGHTS); weight matrices stream as the moving operand. Four concurrent
column-group matmuls (tile_position) cover 4x512 output features at once,
recovering full PE-array width despite the 32-row batch. Per-step
activation transposes ([32,128] -> [128,32] bf16) use the DMA x-bar.
PSUM layout puts gh_n / h / n on partitions 64:128 so every DVE op obeys
the same-space-same-base-partition ISA rule.
"""
import numpy as np
import ml_dtypes

from concourse import bacc, tile, mybir
from concourse.bass_utils import run_bass_kernel_spmd

B, C, H, T = 256, 2048, 1024, 64
G3 = 3 * H  # 3072 gate width
NCORES = 8
BPC = B // NCORES  # 32 batch rows per core
KC = C // 128      # 16 K-tiles over code features
KH = H // 128      # 8 K-tiles over hidden features
NCH_C = C // 512   # 4 output chunks of 512 for C-wide outputs
BF16 = mybir.dt.bfloat16
F32 = mybir.dt.float32

_BUILD_CACHE = {}


def _emit_logits(nc, w_out, lhsT_tiles, logits_ps):
    """logits_ps[32c:32c+32, j] += sum_k lhsT_tiles[k].T @ w_out[:,k,512c+j]"""
    for k in range(KH):
        for c in range(NCH_C):
            nc.tensor.matmul(
                logits_ps[32 * c:32 * c + 32, :],
                lhsT_tiles[:, k, :],
                w_out[:, k, 512 * c:512 * (c + 1)],
                start=(k == 0),
                stop=(k == KH - 1),
                tile_position=(0, 32 * c),
            )


def _emit_codes_epilogue(nc, pools, logits_ps, bias_out, samples_d, t_idx):
    """sigmoid(logits+bias) -> fp32 store to samples[:,t_idx], bf16 cast,
    16 x-bar transposes. Returns codesT tile [128, KC, 32] bf16."""
    sb, tb = pools
    nc.vector.scalar_tensor_tensor(
        logits_ps[:], logits_ps[:], 1.0, bias_out[:],
        mybir.AluOpType.mult, mybir.AluOpType.add)
    codes_f32 = sb.tile([128, 512], F32, tag="codes_f32")
    nc.scalar.activation(codes_f32[:], logits_ps[:],
                         mybir.ActivationFunctionType.Sigmoid)
    nc.sync.dma_start(out=samples_d[:, t_idx, :], in_=codes_f32[:])
    codes_bf = sb.tile([128, 512], BF16, tag="codes_bf")
    nc.vector.tensor_copy(codes_bf[:], codes_f32[:])
    codesT = tb.tile([128, KC, 32], BF16, tag="codesT")
    for j in range(KC):
        c, m = j // 4, j % 4
        nc.sync.dma_start(
            out=codesT[:, j, :],
            in_=codes_bf[32 * c:32 * c + 32, 128 * m:128 * (m + 1)],
            transpose=True)
    return codesT


def _build(t_steps=T):
    if t_steps in _BUILD_CACHE:
        return _BUILD_CACHE[t_steps]
    nc = bacc.Bacc()

    noiseT_d = nc.declare_dram_parameter("noiseT", [128, KH, BPC], BF16, isOutput=False)
    w_ih_d = nc.declare_dram_parameter("w_ih", [128, KC, G3], BF16, isOutput=False)
    w_hh_d = nc.declare_dram_parameter("w_hh", [128, KH, G3], BF16, isOutput=False)
    w_out_d = nc.declare_dram_parameter("w_out", [128, KH, C], BF16, isOutput=False)
    bias_rz_d = nc.declare_dram_parameter("bias_rz", [128, 512], F32, isOutput=False)
    bias_n_d = nc.declare_dram_parameter("bias_n", [128, 512], F32, isOutput=False)
    bias_out_d = nc.declare_dram_parameter("bias_out", [128, 512], F32, isOutput=False)
    # packed layouts: line index = 32*chunk + batch_row
    samples_d = nc.declare_dram_parameter("samples", [128, t_steps, 512], F32, isOutput=True)
    hiddens_d = nc.declare_dram_parameter("hiddens", [64, t_steps, 512], F32, isOutput=True)

    with tile.TileContext(nc) as tc:
        with (
            tc.tile_pool(name="wpool", bufs=1) as wp,
            tc.tile_pool(name="spool", bufs=2) as sb,
            tc.tile_pool(name="spool1", bufs=1) as sb1,
            tc.tile_pool(name="tpool", bufs=2) as tb,
            tc.tile_pool(name="psum", bufs=2, space="PSUM") as ps,
        ):
            w_ih = wp.tile([128, KC, G3], BF16)
            w_hh = wp.tile([128, KH, G3], BF16)
            w_out = wp.tile([128, KH, C], BF16)
            bias_rz = wp.tile([128, 512], F32)
            bias_n = wp.tile([128, 512], F32)
            bias_out = wp.tile([128, 512], F32)
            noiseT = wp.tile([128, KH, BPC], BF16)
            nc.sync.dma_start(w_out[:], w_out_d[:])
            nc.sync.dma_start(w_hh[:], w_hh_d[:])
            nc.sync.dma_start(w_ih[:], w_ih_d[:])
            nc.sync.dma_start(bias_rz[:], bias_rz_d[:])
            nc.sync.dma_start(bias_n[:], bias_n_d[:])
            nc.sync.dma_start(bias_out[:], bias_out_d[:])
            nc.sync.dma_start(noiseT[:], noiseT_d[:])

            # ---- init: codes0 = sigmoid(noise @ W_out.T + b_out), h0 = 0
            logits_ps = ps.tile([128, 512], F32, tag="logits")
            _emit_logits(nc, w_out, noiseT, logits_ps)
            codesT = _emit_codes_epilogue(nc, (sb, tb), logits_ps, bias_out,
                                          samples_d, 0)
            h_cur = sb.tile([128, 512], F32, tag="h")   # h on partitions 64:128
            nc.vector.memset(h_cur[:], 0.0)
            hT = tb.tile([128, KH, BPC], BF16, tag="hT")
            nc.vector.memset(hT[:], 0.0)

            for t in range(t_steps):
                # ---- gates pass 1: r|z chunks (cols 0:2048 of the 3H gates)
                # col group c <- gate cols 512c.  gi+gh accumulate together.
                rz_ps = ps.tile([128, 512], F32, tag="rz")
                n_iter = KH + KC
                for i in range(n_iter):  # h-tiles first (available earlier)
                    if i < KH:
                        lhsT, w, k = hT[:, i, :], w_hh, i
                    else:
                        lhsT, w, k = codesT[:, i - KH, :], w_ih, i - KH
                    for c in range(4):
                        nc.tensor.matmul(
                            rz_ps[32 * c:32 * c + 32, :],
                            lhsT, w[:, k, 512 * c:512 * (c + 1)],
                            start=(i == 0), stop=(i == n_iter - 1),
                            tile_position=(0, 32 * c))
                # ---- gates pass 2: n chunks (cols 2048:3072)
                # gi_n -> col groups {0,1} (partitions 0:64)
                # gh_n -> col groups {2,3} (partitions 64:128)
                n_ps = ps.tile([128, 512], F32, tag="n")
                for k in range(KH):
                    for c in range(2):
                        nc.tensor.matmul(
                            n_ps[64 + 32 * c:96 + 32 * c, :],
                            hT[:, k, :], w_hh[:, k, 2048 + 512 * c:2048 + 512 * (c + 1)],
                            start=(k == 0), stop=(k == KH - 1),
                            tile_position=(0, 64 + 32 * c))
                for k in range(KC):
                    for c in range(2):
                        nc.tensor.matmul(
                            n_ps[32 * c:32 * c + 32, :],
                            codesT[:, k, :], w_ih[:, k, 2048 + 512 * c:2048 + 512 * (c + 1)],
                            start=(k == 0), stop=(k == KC - 1),
                            tile_position=(0, 32 * c))

                # ---- epilogue
                # r/z: sigmoid(rz + b).  r -> rz_sb[0:64], z -> rz_sb[64:128]
                nc.vector.scalar_tensor_tensor(
                    rz_ps[:], rz_ps[:], 1.0, bias_rz[:],
                    mybir.AluOpType.mult, mybir.AluOpType.add)
                rz_sb = sb1.tile([128, 512], F32, tag="rz_sb")
                nc.scalar.activation(rz_sb[:], rz_ps[:],
                                     mybir.ActivationFunctionType.Sigmoid)
                # ghn' = gh_n + b_hh_n   (psum upper half, in place)
                nc.vector.scalar_tensor_tensor(
                    n_ps[64:128, :], n_ps[64:128, :], 1.0, bias_n[64:128, :],
                    mybir.AluOpType.mult, mybir.AluOpType.add)
                # s = ghn' * r   (in place upper; r is SBUF@0 - legal mixed-space)
                nc.vector.scalar_tensor_tensor(
                    n_ps[64:128, :], n_ps[64:128, :], 1.0, rz_sb[0:64, :],
                    mybir.AluOpType.mult, mybir.AluOpType.mult)
                # u = gi_n + b_ih_n -> SBUF@0 (lives in n_sb's unused lower half)
                n_sb = sb1.tile([128, 512], F32, tag="n_sb")
                nc.vector.scalar_tensor_tensor(
                    n_sb[0:64, :], n_ps[0:64, :], 1.0, bias_n[0:64, :],
                    mybir.AluOpType.mult, mybir.AluOpType.add)
                # npre = s + u  (in place upper psum; u is SBUF@0)
                nc.vector.scalar_tensor_tensor(
                    n_ps[64:128, :], n_ps[64:128, :], 1.0, n_sb[0:64, :],
                    mybir.AluOpType.mult, mybir.AluOpType.add)
                # n = tanh(npre) -> SBUF@64
                nc.scalar.activation(n_sb[64:128, :], n_ps[64:128, :],
                                     mybir.ActivationFunctionType.Tanh)
                # d = h - n ; e = d * z ; h' = n + e   (all @64)
                d_sb = sb1.tile([128, 512], F32, tag="d_sb")
                nc.vector.scalar_tensor_tensor(
                    d_sb[64:128, :], h_cur[64:128, :], 1.0, n_sb[64:128, :],
                    mybir.AluOpType.mult, mybir.AluOpType.subtract)
                nc.vector.scalar_tensor_tensor(
                    d_sb[64:128, :], d_sb[64:128, :], 1.0, rz_sb[64:128, :],
                    mybir.AluOpType.mult, mybir.AluOpType.mult)
                h_new = sb.tile([128, 512], F32, tag="h")
                nc.vector.scalar_tensor_tensor(
                    h_new[64:128, :], d_sb[64:128, :], 1.0, n_sb[64:128, :],
                    mybir.AluOpType.mult, mybir.AluOpType.add)
                # store hiddens[:, t] = h'
                nc.sync.dma_start(out=hiddens_d[:, t, :], in_=h_new[64:128, :])
                # bf16 cast + 8 x-bar transposes -> hT for next step / logits
                h_bf = sb1.tile([128, 512], BF16, tag="h_bf")
                nc.vector.tensor_copy(h_bf[64:128, :], h_new[64:128, :])
                hT = tb.tile([128, KH, BPC], BF16, tag="hT")
                for j in range(KH):
                    c, m = j // 4, j % 4
                    nc.sync.dma_start(
                        out=hT[:, j, :],
                        in_=h_bf[64 + 32 * c:96 + 32 * c, 128 * m:128 * (m + 1)],
                        transpose=True)
                h_cur = h_new

                # ---- logits -> codes for next step (skip on last)
                if t < t_steps - 1:
                    logits_ps = ps.tile([128, 512], F32, tag="logits")
                    _emit_logits(nc, w_out, hT, logits_ps)
                    codesT = _emit_codes_epilogue(nc, (sb, tb), logits_ps,
                                                  bias_out, samples_d, t + 1)

    nc.finalize()
    _BUILD_CACHE[t_steps] = nc
    return nc


def _pack_inputs(noise, W_ih, b_ih, W_hh, b_hh, W_out, b_out, t_steps=T):
    bf = ml_dtypes.bfloat16
    w_ih = np.ascontiguousarray(
        W_ih.T.astype(bf).reshape(KC, 128, G3).transpose(1, 0, 2))
    w_hh = np.ascontiguousarray(
        W_hh.T.astype(bf).reshape(KH, 128, G3).transpose(1, 0, 2))
    w_out = np.ascontiguousarray(
        W_out.T.astype(bf).reshape(KH, 128, C).transpose(1, 0, 2))
    brz = np.ascontiguousarray(np.broadcast_to(
        (b_ih + b_hh)[:2048].reshape(4, 1, 512), (4, 32, 512))
    ).reshape(128, 512).astype(np.float32)
    bn_lo = np.broadcast_to(b_ih[2048:].reshape(2, 1, 512), (2, 32, 512)).reshape(64, 512)
    bn_hi = np.broadcast_to(b_hh[2048:].reshape(2, 1, 512), (2, 32, 512)).reshape(64, 512)
    bn = np.ascontiguousarray(np.concatenate([bn_lo, bn_hi], 0)).astype(np.float32)
    bout = np.ascontiguousarray(np.broadcast_to(
        b_out.reshape(4, 1, 512), (4, 32, 512))).reshape(128, 512).astype(np.float32)

    shared = {"w_ih": w_ih, "w_hh": w_hh, "w_out": w_out,
              "bias_rz": brz, "bias_n": bn, "bias_out": bout}
    in_maps = []
    for i in range(NCORES):
        noiseT = np.ascontiguousarray(
            noise[BPC * i:BPC * (i + 1)].T.astype(bf)
            .reshape(KH, 128, BPC).transpose(1, 0, 2))
        in_maps.append({**shared, "noiseT": noiseT})
    return in_maps


def _run(noise, W_ih, b_ih, W_hh, b_hh, W_out, b_out, t_steps=T, **spmd_kwargs):
    nc = _build(t_steps)
    in_maps = _pack_inputs(noise, W_ih, b_ih, W_hh, b_hh, W_out, b_out, t_steps)
    res = run_bass_kernel_spmd(nc, in_maps, list(range(NCORES)), **spmd_kwargs)

    def unpack(name, nch, width):
        per_core = []
        for i in range(NCORES):
            p = res.results[i][name]  # [32*nch, t, 512] packed
            t_n = p.shape[1]
            per_core.append(
                p.reshape(nch, BPC, t_n, 512).transpose(1, 2, 0, 3)
                .reshape(BPC, t_n, width))
        return np.concatenate(per_core, 0)

    samples = unpack("samples", 4, C)
    hiddens = unpack("hiddens", 2, H)
    return (samples, hiddens), res


def kernel(noise, W_ih, b_ih, W_hh, b_hh, W_out, b_out, max_len):
    assert int(max_len) == T, f"kernel hardcodes T={T}, got {max_len}"
    noise = np.asarray(noise, dtype=np.float32)
    (samples, hiddens), _ = _run(
        noise, np.asarray(W_ih, np.float32), np.asarray(b_ih, np.float32),
        np.asarray(W_hh, np.float32), np.asarray(b_hh, np.float32),
        np.asarray(W_out, np.float32), np.asarray(b_out, np.float32))
    return samples, hiddens


# revision 12
# speedup vs baseline: 1.4326x; 1.4326x over previous
"""GRU sampler kernel for Trainium2 (8 NeuronCores, batch-data-parallel).

Reference computation (per batch row, T=64 steps):
    codes0 = sigmoid(noise @ W_out.T + b_out)
    per step: gi = codes @ W_ih.T + b_ih ; gh = h @ W_hh.T + b_hh
              r = sig(gi_r + gh_r); z = sig(gi_z + gh_z)
              n = tanh(gi_n + r * gh_n); h' = (1-z)*n + z*h
              codes' = sigmoid(h' @ W_out.T + b_out)
    samples[t] = codes (pre-cell), hiddens[t] = h' (post-cell)

Strategy: batch 256 split 8 ways (32/core). Weights cast to bf16 and kept
resident in SBUF (fp32 would not fit). Activations are the matmul
*stationary* operand (feature-on-partition, only 32 columns -> cheap
LDWEIGHTS); weight matrices stream as the moving operand. Four concurrent
column-group matmuls (tile_position) cover 4x512 output features at once,
recovering full PE-array width despite the 32-row batch. Per-step
activation transposes ([32,128] -> [128,32] bf16) use the DMA x-bar.
PSUM layout puts gh_n / h / n on partitions 64:128 so every DVE op obeys
the same-space-same-base-partition ISA rule.
"""
import numpy as np
import ml_dtypes

from concourse import bacc, tile, mybir
from concourse.bass_utils import run_bass_kernel_spmd

B, C, H, T = 256, 2048, 1024, 64
G3 = 3 * H  # 3072 gate width
NCORES = 8
BPC = B // NCORES  # 32 batch rows per core
KC = C // 128      # 16 K-tiles over code features
KH = H // 128      # 8 K-tiles over hidden features
NCH_C = C // 512   # 4 output chunks of 512 for C-wide outputs
BF16 = mybir.dt.bfloat16
F32 = mybir.dt.float32

_BUILD_CACHE = {}


def _emit_logits(nc, w_out, lhsT_tiles, logits_ps):
    """logits_ps[32c:32c+32, j] += sum_k lhsT_tiles[k].T @ w_out[:,k,512c+j]"""
    for k in range(KH):
        for c in range(NCH_C):
            nc.tensor.matmul(
                logits_ps[32 * c:32 * c + 32, :],
                lhsT_tiles[:, k, :],
                w_out[:, k, 512 * c:512 * (c + 1)],
                start=(k == 0),
                stop=(k == KH - 1),
                tile_position=(0, 32 * c),
            )


def _emit_codes_epilogue(nc, pools, logits_ps, bias_out, samples_d, t_idx):
    """sigmoid(logits+bias) -> fp32 store to samples[:,t_idx], bf16 cast,
    16 x-bar transposes. Returns codesT tile [128, KC, 32] bf16."""
    sb, tb = pools
    nc.vector.scalar_tensor_tensor(
        logits_ps[:], logits_ps[:], 1.0, bias_out[:],
        mybir.AluOpType.mult, mybir.AluOpType.add)
    codes_f32 = sb.tile([128, 512], F32, tag="codes_f32")
    nc.scalar.activation(codes_f32[:], logits_ps[:],
                         mybir.ActivationFunctionType.Sigmoid)
    nc.scalar.dma_start(out=samples_d[:, t_idx, :], in_=codes_f32[:])
    codes_bf = sb.tile([128, 512], BF16, tag="codes_bf")
    nc.vector.tensor_copy(codes_bf[:], codes_f32[:])
    codesT = tb.tile([128, KC, 32], BF16, tag="codesT")
    for c in range(4):  # batched x-bar transpose: [32,512] -> [128,4,32]
        eng = nc.sync if c % 2 == 0 else nc.scalar
        eng.dma_start(
            out=codesT[:, 4 * c:4 * (c + 1), :],
            in_=codes_bf[32 * c:32 * c + 32, :],
            transpose=True)
    return codesT


def _build(t_steps=T):
    if t_steps in _BUILD_CACHE:
        return _BUILD_CACHE[t_steps]
    nc = bacc.Bacc()

    noiseT_d = nc.declare_dram_parameter("noiseT", [128, KH, BPC], BF16, isOutput=False)
    w_ih_d = nc.declare_dram_parameter("w_ih", [128, KC, G3], BF16, isOutput=False)
    w_hh_d = nc.declare_dram_parameter("w_hh", [128, KH, G3], BF16, isOutput=False)
    w_out_d = nc.declare_dram_parameter("w_out", [128, KH, C], BF16, isOutput=False)
    bias_rz_d = nc.declare_dram_parameter("bias_rz", [128, 512], F32, isOutput=False)
    bias_n_d = nc.declare_dram_parameter("bias_n", [128, 512], F32, isOutput=False)
    bias_out_d = nc.declare_dram_parameter("bias_out", [128, 512], F32, isOutput=False)
    # packed layouts: line index = 32*chunk + batch_row
    samples_d = nc.declare_dram_parameter("samples", [128, t_steps, 512], F32, isOutput=True)
    hiddens_d = nc.declare_dram_parameter("hiddens", [64, t_steps, 512], F32, isOutput=True)

    with tile.TileContext(nc) as tc:
        with (
            tc.tile_pool(name="wpool", bufs=1) as wp,
            tc.tile_pool(name="spool", bufs=2) as sb,
            tc.tile_pool(name="spool1", bufs=1) as sb1,
            tc.tile_pool(name="tpool", bufs=2) as tb,
            tc.tile_pool(name="psum", bufs=2, space="PSUM") as ps,
        ):
            w_ih = wp.tile([128, KC, G3], BF16)
            w_hh = wp.tile([128, KH, G3], BF16)
            w_out = wp.tile([128, KH, C], BF16)
            bias_rz = wp.tile([128, 512], F32)
            bias_n = wp.tile([128, 512], F32)
            bias_out = wp.tile([128, 512], F32)
            noiseT = wp.tile([128, KH, BPC], BF16)
            nc.sync.dma_start(w_out[:], w_out_d[:])
            nc.sync.dma_start(w_hh[:], w_hh_d[:])
            nc.sync.dma_start(w_ih[:], w_ih_d[:])
            nc.sync.dma_start(bias_rz[:], bias_rz_d[:])
            nc.sync.dma_start(bias_n[:], bias_n_d[:])
            nc.sync.dma_start(bias_out[:], bias_out_d[:])
            nc.sync.dma_start(noiseT[:], noiseT_d[:])

            # ---- init: codes0 = sigmoid(noise @ W_out.T + b_out), h0 = 0
            logits_ps = ps.tile([128, 512], F32, tag="logits")
            _emit_logits(nc, w_out, noiseT, logits_ps)
            codesT = _emit_codes_epilogue(nc, (sb, tb), logits_ps, bias_out,
                                          samples_d, 0)
            h_cur = sb.tile([128, 512], F32, tag="h")   # h on partitions 64:128
            nc.vector.memset(h_cur[:], 0.0)
            hT = tb.tile([128, KH, BPC], BF16, tag="hT")
            nc.vector.memset(hT[:], 0.0)

            for t in range(t_steps):
                # ---- gates pass 1: r|z chunks (cols 0:2048 of the 3H gates)
                # col group c <- gate cols 512c.  gi+gh accumulate together.
                rz_ps = ps.tile([128, 512], F32, tag="rz")
                n_iter = KH + KC
                for i in range(n_iter):  # h-tiles first (available earlier)
                    if i < KH:
                        lhsT, w, k = hT[:, i, :], w_hh, i
                    else:
                        lhsT, w, k = codesT[:, i - KH, :], w_ih, i - KH
                    for c in range(4):
                        nc.tensor.matmul(
                            rz_ps[32 * c:32 * c + 32, :],
                            lhsT, w[:, k, 512 * c:512 * (c + 1)],
                            start=(i == 0), stop=(i == n_iter - 1),
                            tile_position=(0, 32 * c))
                # ---- gates pass 2: n chunks (cols 2048:3072)
                # gi_n -> col groups {0,1} (partitions 0:64)
                # gh_n -> col groups {2,3} (partitions 64:128)
                n_ps = ps.tile([128, 512], F32, tag="n")
                for k in range(KH):
                    for c in range(2):
                        nc.tensor.matmul(
                            n_ps[64 + 32 * c:96 + 32 * c, :],
                            hT[:, k, :], w_hh[:, k, 2048 + 512 * c:2048 + 512 * (c + 1)],
                            start=(k == 0), stop=(k == KH - 1),
                            tile_position=(0, 64 + 32 * c))
                for k in range(KC):
                    for c in range(2):
                        nc.tensor.matmul(
                            n_ps[32 * c:32 * c + 32, :],
                            codesT[:, k, :], w_ih[:, k, 2048 + 512 * c:2048 + 512 * (c + 1)],
                            start=(k == 0), stop=(k == KC - 1),
                            tile_position=(0, 32 * c))

                # ---- epilogue
                # r/z: sigmoid(rz + b).  r -> rz_sb[0:64], z -> rz_sb[64:128]
                nc.vector.scalar_tensor_tensor(
                    rz_ps[:], rz_ps[:], 1.0, bias_rz[:],
                    mybir.AluOpType.mult, mybir.AluOpType.add)
                rz_sb = sb1.tile([128, 512], F32, tag="rz_sb")
                nc.scalar.activation(rz_sb[:], rz_ps[:],
                                     mybir.ActivationFunctionType.Sigmoid)
                # ghn' = gh_n + b_hh_n   (psum upper half, in place)
                nc.vector.scalar_tensor_tensor(
                    n_ps[64:128, :], n_ps[64:128, :], 1.0, bias_n[64:128, :],
                    mybir.AluOpType.mult, mybir.AluOpType.add)
                # s = ghn' * r   (in place upper; r is SBUF@0 - legal mixed-space)
                nc.vector.scalar_tensor_tensor(
                    n_ps[64:128, :], n_ps[64:128, :], 1.0, rz_sb[0:64, :],
                    mybir.AluOpType.mult, mybir.AluOpType.mult)
                # u = gi_n + b_ih_n -> SBUF@0 (lives in n_sb's unused lower half)
                n_sb = sb1.tile([128, 512], F32, tag="n_sb")
                nc.vector.scalar_tensor_tensor(
                    n_sb[0:64, :], n_ps[0:64, :], 1.0, bias_n[0:64, :],
                    mybir.AluOpType.mult, mybir.AluOpType.add)
                # npre = s + u  (in place upper psum; u is SBUF@0)
                nc.vector.scalar_tensor_tensor(
                    n_ps[64:128, :], n_ps[64:128, :], 1.0, n_sb[0:64, :],
                    mybir.AluOpType.mult, mybir.AluOpType.add)
                # n = tanh(npre) -> SBUF@64
                nc.scalar.activation(n_sb[64:128, :], n_ps[64:128, :],
                                     mybir.ActivationFunctionType.Tanh)
                # d = h - n ; e = d * z ; h' = n + e   (all @64)
                d_sb = sb1.tile([128, 512], F32, tag="d_sb")
                nc.vector.scalar_tensor_tensor(
                    d_sb[64:128, :], h_cur[64:128, :], 1.0, n_sb[64:128, :],
                    mybir.AluOpType.mult, mybir.AluOpType.subtract)
                nc.vector.scalar_tensor_tensor(
                    d_sb[64:128, :], d_sb[64:128, :], 1.0, rz_sb[64:128, :],
                    mybir.AluOpType.mult, mybir.AluOpType.mult)
                h_new = sb.tile([128, 512], F32, tag="h")
                nc.vector.scalar_tensor_tensor(
                    h_new[64:128, :], d_sb[64:128, :], 1.0, n_sb[64:128, :],
                    mybir.AluOpType.mult, mybir.AluOpType.add)
                # store hiddens[:, t] = h'
                nc.scalar.dma_start(out=hiddens_d[:, t, :], in_=h_new[64:128, :])
                # bf16 cast + batched x-bar transposes -> hT
                h_bf = sb1.tile([128, 512], BF16, tag="h_bf")
                nc.vector.tensor_copy(h_bf[64:128, :], h_new[64:128, :])
                hT = tb.tile([128, KH, BPC], BF16, tag="hT")
                for c in range(2):
                    eng = nc.sync if c % 2 == 0 else nc.scalar
                    eng.dma_start(
                        out=hT[:, 4 * c:4 * (c + 1), :],
                        in_=h_bf[64 + 32 * c:96 + 32 * c, :],
                        transpose=True)
                h_cur = h_new

                # ---- logits -> codes for next step (skip on last)
                if t < t_steps - 1:
                    logits_ps = ps.tile([128, 512], F32, tag="logits")
                    _emit_logits(nc, w_out, hT, logits_ps)
                    codesT = _emit_codes_epilogue(nc, (sb, tb), logits_ps,
                                                  bias_out, samples_d, t + 1)

    nc.finalize()
    _BUILD_CACHE[t_steps] = nc
    return nc


def _pack_inputs(noise, W_ih, b_ih, W_hh, b_hh, W_out, b_out, t_steps=T):
    bf = ml_dtypes.bfloat16
    w_ih = np.ascontiguousarray(
        W_ih.T.astype(bf).reshape(KC, 128, G3).transpose(1, 0, 2))
    w_hh = np.ascontiguousarray(
        W_hh.T.astype(bf).reshape(KH, 128, G3).transpose(1, 0, 2))
    w_out = np.ascontiguousarray(
        W_out.T.astype(bf).reshape(KH, 128, C).transpose(1, 0, 2))
    brz = np.ascontiguousarray(np.broadcast_to(
        (b_ih + b_hh)[:2048].reshape(4, 1, 512), (4, 32, 512))
    ).reshape(128, 512).astype(np.float32)
    bn_lo = np.broadcast_to(b_ih[2048:].reshape(2, 1, 512), (2, 32, 512)).reshape(64, 512)
    bn_hi = np.broadcast_to(b_hh[2048:].reshape(2, 1, 512), (2, 32, 512)).reshape(64, 512)
    bn = np.ascontiguousarray(np.concatenate([bn_lo, bn_hi], 0)).astype(np.float32)
    bout = np.ascontiguousarray(np.broadcast_to(
        b_out.reshape(4, 1, 512), (4, 32, 512))).reshape(128, 512).astype(np.float32)

    shared = {"w_ih": w_ih, "w_hh": w_hh, "w_out": w_out,
              "bias_rz": brz, "bias_n": bn, "bias_out": bout}
    in_maps = []
    for i in range(NCORES):
        noiseT = np.ascontiguousarray(
            noise[BPC * i:BPC * (i + 1)].T.astype(bf)
            .reshape(KH, 128, BPC).transpose(1, 0, 2))
        in_maps.append({**shared, "noiseT": noiseT})
    return in_maps


def _run(noise, W_ih, b_ih, W_hh, b_hh, W_out, b_out, t_steps=T, **spmd_kwargs):
    nc = _build(t_steps)
    in_maps = _pack_inputs(noise, W_ih, b_ih, W_hh, b_hh, W_out, b_out, t_steps)
    res = run_bass_kernel_spmd(nc, in_maps, list(range(NCORES)), **spmd_kwargs)

    def unpack(name, nch, width):
        per_core = []
        for i in range(NCORES):
            p = res.results[i][name]  # [32*nch, t, 512] packed
            t_n = p.shape[1]
            per_core.append(
                p.reshape(nch, BPC, t_n, 512).transpose(1, 2, 0, 3)
                .reshape(BPC, t_n, width))
        return np.concatenate(per_core, 0)

    samples = unpack("samples", 4, C)
    hiddens = unpack("hiddens", 2, H)
    return (samples, hiddens), res


def kernel(noise, W_ih, b_ih, W_hh, b_hh, W_out, b_out, max_len):
    assert int(max_len) == T, f"kernel hardcodes T={T}, got {max_len}"
    noise = np.asarray(noise, dtype=np.float32)
    (samples, hiddens), _ = _run(
        noise, np.asarray(W_ih, np.float32), np.asarray(b_ih, np.float32),
        np.asarray(W_hh, np.float32), np.asarray(b_hh, np.float32),
        np.asarray(W_out, np.float32), np.asarray(b_out, np.float32))
    return samples, hiddens


# revision 13
# speedup vs baseline: 1.6976x; 1.1850x over previous
"""GRU sampler kernel for Trainium2 (8 NeuronCores, batch-data-parallel).

Reference computation (per batch row, T=64 steps):
    codes0 = sigmoid(noise @ W_out.T + b_out)
    per step: gi = codes @ W_ih.T + b_ih ; gh = h @ W_hh.T + b_hh
              r = sig(gi_r + gh_r); z = sig(gi_z + gh_z)
              n = tanh(gi_n + r * gh_n); h' = (1-z)*n + z*h
              codes' = sigmoid(h' @ W_out.T + b_out)
    samples[t] = codes (pre-cell), hiddens[t] = h' (post-cell)

Strategy: batch 256 split 8 ways (32/core). Weights cast to bf16 and kept
resident in SBUF (fp32 would not fit). Activations are the matmul
*stationary* operand (feature-on-partition, only 32 columns -> cheap
LDWEIGHTS); weight matrices stream as the moving operand. Four concurrent
column-group matmuls (tile_position) cover 4x512 output features at once,
recovering full PE-array width despite the 32-row batch. Biases enter via
K=1 ones-matmuls that seed each PSUM accumulation. Per-step activation
transposes ([32,512] -> [128,4,32] bf16) use the DMA x-bar. The epilogue
is column-chunked so the serial DVE/ACT chain pipelines. PSUM layout puts
gh_n / h / n on partitions 64:128 so every DVE op obeys the
same-space-same-base-partition ISA rule.
"""
import numpy as np
import ml_dtypes

from concourse import bacc, tile, mybir
from concourse.bass_utils import run_bass_kernel_spmd

B, C, H, T = 256, 2048, 1024, 64
G3 = 3 * H  # 3072 gate width
NCORES = 8
BPC = B // NCORES  # 32 batch rows per core
KC = C // 128      # 16 K-tiles over code features
KH = H // 128      # 8 K-tiles over hidden features
EP_CH = 2          # epilogue column chunks
ECW = 512 // EP_CH
BF16 = mybir.dt.bfloat16
F32 = mybir.dt.float32
MULT = None
ADD = None

_BUILD_CACHE = {}


def _emit_logits(nc, ones, brow, w_out, lhsT_tiles, logits_ps):
    """logits_ps[32c:32c+32, j] = b_out + sum_k lhsT[k].T @ w_out[:,k,512c+j]"""
    for c in range(4):
        nc.tensor.matmul(
            logits_ps[32 * c:32 * c + 32, :], ones[:],
            brow[0:1, 512 * c:512 * (c + 1)],
            start=True, stop=False, tile_position=(0, 32 * c))
    for k in range(KH):
        for c in range(4):
            nc.tensor.matmul(
                logits_ps[32 * c:32 * c + 32, :],
                lhsT_tiles[:, k, :],
                w_out[:, k, 512 * c:512 * (c + 1)],
                start=False, stop=(k == KH - 1),
                tile_position=(0, 32 * c))


def _emit_codes_epilogue(nc, pools, logits_ps, samples_d, t_idx):
    """sigmoid(logits) -> bf16 (chain) + fp32 copy for the samples store;
    4 batched x-bar transposes. Returns codesT [128, KC, 32] bf16."""
    sb, sb1, tb = pools
    codes_bf = sb1.tile([128, 512], BF16, tag="codes_bf")
    codes_f32 = sb1.tile([128, 512], F32, tag="codes_f32")
    for ch in range(EP_CH):
        cs = slice(ECW * ch, ECW * (ch + 1))
        nc.scalar.activation(codes_bf[:, cs], logits_ps[:, cs],
                             mybir.ActivationFunctionType.Sigmoid)
        nc.vector.tensor_copy(codes_f32[:, cs], codes_bf[:, cs])
    nc.scalar.dma_start(out=samples_d[:, t_idx, :], in_=codes_f32[:])
    codesT = tb.tile([128, KC, 32], BF16, tag="codesT")
    for c in range(4):  # batched x-bar transpose: [32,512] -> [128,4,32]
        eng = nc.sync if c % 2 == 0 else nc.scalar
        eng.dma_start(
            out=codesT[:, 4 * c:4 * (c + 1), :],
            in_=codes_bf[32 * c:32 * c + 32, :],
            transpose=True)
    return codesT


def _build(t_steps=T):
    if t_steps in _BUILD_CACHE:
        return _BUILD_CACHE[t_steps]
    global MULT, ADD
    MULT = mybir.AluOpType.mult
    ADD = mybir.AluOpType.add
    nc = bacc.Bacc()

    noiseT_d = nc.declare_dram_parameter("noiseT", [128, KH, BPC], BF16, isOutput=False)
    w_ih_d = nc.declare_dram_parameter("w_ih", [128, KC, G3], BF16, isOutput=False)
    w_hh_d = nc.declare_dram_parameter("w_hh", [128, KH, G3], BF16, isOutput=False)
    w_out_d = nc.declare_dram_parameter("w_out", [128, KH, C], BF16, isOutput=False)
    # bias rows for the K=1 ones-matmuls (bf16, matching the weight dtype)
    brz_d = nc.declare_dram_parameter("bias_rz", [1, 2048], BF16, isOutput=False)
    bn_d = nc.declare_dram_parameter("bias_n", [1, 2048], BF16, isOutput=False)
    bout_d = nc.declare_dram_parameter("bias_out", [1, 2048], BF16, isOutput=False)
    # packed layouts: line index = 32*chunk + batch_row
    samples_d = nc.declare_dram_parameter("samples", [128, t_steps, 512], F32, isOutput=True)
    hiddens_d = nc.declare_dram_parameter("hiddens", [64, t_steps, 512], F32, isOutput=True)

    with tile.TileContext(nc) as tc:
        with (
            tc.tile_pool(name="wpool", bufs=1) as wp,
            tc.tile_pool(name="spool", bufs=2) as sb,
            tc.tile_pool(name="spool1", bufs=1) as sb1,
            tc.tile_pool(name="tpool", bufs=2) as tb,
            tc.tile_pool(name="psum", bufs=2, space="PSUM") as ps,
        ):
            noiseT = wp.tile([128, KH, BPC], BF16)
            brz = wp.tile([1, 2048], BF16)
            bn = wp.tile([1, 2048], BF16)
            bout = wp.tile([1, 2048], BF16)
            w_ih = wp.tile([128, KC, G3], BF16)
            w_hh = wp.tile([128, KH, G3], BF16)
            w_out = wp.tile([128, KH, C], BF16)
            nc.sync.dma_start(noiseT[:], noiseT_d[:])
            nc.sync.dma_start(brz[:], brz_d[:])
            nc.sync.dma_start(bn[:], bn_d[:])
            nc.sync.dma_start(bout[:], bout_d[:])
            nc.sync.dma_start(w_out[:], w_out_d[:])
            nc.sync.dma_start(w_hh[:], w_hh_d[:])
            nc.sync.dma_start(w_ih[:], w_ih_d[:])
            ones = wp.tile([1, BPC], BF16)
            nc.vector.memset(ones[:], 1.0)

            # ---- init: codes0 = sigmoid(noise @ W_out.T + b_out), h0 = 0
            logits_ps = ps.tile([128, 512], F32, tag="logits")
            _emit_logits(nc, ones, bout, w_out, noiseT, logits_ps)
            codesT = _emit_codes_epilogue(nc, (sb, sb1, tb), logits_ps,
                                          samples_d, 0)
            h_cur = sb.tile([128, 512], F32, tag="h")   # h on partitions 64:128
            nc.vector.memset(h_cur[:], 0.0)
            hT = tb.tile([128, KH, BPC], BF16, tag="hT")
            nc.vector.memset(hT[:], 0.0)

            for t in range(t_steps):
                # ---- gates pass 1: r|z chunks (cols 0:2048 of the 3H gates)
                rz_ps = ps.tile([128, 512], F32, tag="rz")
                for c in range(4):
                    nc.tensor.matmul(
                        rz_ps[32 * c:32 * c + 32, :], ones[:],
                        brz[0:1, 512 * c:512 * (c + 1)],
                        start=True, stop=False, tile_position=(0, 32 * c))
                n_iter = KH + KC
                for i in range(n_iter):  # h-tiles first (available earlier)
                    if i < KH:
                        lhsT, w, k = hT[:, i, :], w_hh, i
                    else:
                        lhsT, w, k = codesT[:, i - KH, :], w_ih, i - KH
                    for c in range(4):
                        nc.tensor.matmul(
                            rz_ps[32 * c:32 * c + 32, :],
                            lhsT, w[:, k, 512 * c:512 * (c + 1)],
                            start=False, stop=(i == n_iter - 1),
                            tile_position=(0, 32 * c))
                # ---- gates pass 2: n chunks (cols 2048:3072)
                # gh_n -> col groups {2,3} (partitions 64:128), first
                # gi_n -> col groups {0,1} (partitions 0:64)
                n_ps = ps.tile([128, 512], F32, tag="n")
                for c in range(2):
                    nc.tensor.matmul(
                        n_ps[64 + 32 * c:96 + 32 * c, :], ones[:],
                        bn[0:1, 1024 + 512 * c:1024 + 512 * (c + 1)],
                        start=True, stop=False, tile_position=(0, 64 + 32 * c))
                    nc.tensor.matmul(
                        n_ps[32 * c:32 * c + 32, :], ones[:],
                        bn[0:1, 512 * c:512 * (c + 1)],
                        start=True, stop=False, tile_position=(0, 32 * c))
                for k in range(KH):
                    for c in range(2):
                        nc.tensor.matmul(
                            n_ps[64 + 32 * c:96 + 32 * c, :],
                            hT[:, k, :], w_hh[:, k, 2048 + 512 * c:2048 + 512 * (c + 1)],
                            start=False, stop=(k == KH - 1),
                            tile_position=(0, 64 + 32 * c))
                for k in range(KC):
                    for c in range(2):
                        nc.tensor.matmul(
                            n_ps[32 * c:32 * c + 32, :],
                            codesT[:, k, :], w_ih[:, k, 2048 + 512 * c:2048 + 512 * (c + 1)],
                            start=False, stop=(k == KC - 1),
                            tile_position=(0, 32 * c))

                # ---- epilogue (column-chunked so the serial chain pipelines)
                rz_sb = sb1.tile([128, 512], F32, tag="rz_sb")
                n_sb = sb1.tile([128, 512], F32, tag="n_sb")
                d_sb = sb1.tile([128, 512], F32, tag="d_sb")
                h_bf = sb1.tile([128, 512], BF16, tag="h_bf")
                h_new = sb.tile([128, 512], F32, tag="h")
                for ch in range(EP_CH):
                    cs = slice(ECW * ch, ECW * (ch + 1))
                    # r/z = sigmoid(rz)
                    nc.scalar.activation(rz_sb[:, cs], rz_ps[:, cs],
                                         mybir.ActivationFunctionType.Sigmoid)
                    # s = ghn * r  (in place upper psum; r is SBUF@0)
                    nc.vector.scalar_tensor_tensor(
                        n_ps[64:128, cs], n_ps[64:128, cs], 1.0, rz_sb[0:64, cs],
                        MULT, MULT)
                    # u = gi_n -> SBUF@0 (off-chain wrt s)
                    nc.vector.tensor_copy(n_sb[0:64, cs], n_ps[0:64, cs])
                    # npre = s + u
                    nc.vector.scalar_tensor_tensor(
                        n_ps[64:128, cs], n_ps[64:128, cs], 1.0, n_sb[0:64, cs],
                        MULT, ADD)
                    # n = tanh(npre) -> SBUF@64
                    nc.scalar.activation(n_sb[64:128, cs], n_ps[64:128, cs],
                                         mybir.ActivationFunctionType.Tanh)
                    # d = h - n ; e = d * z ; h' = n + e (bf16 chain + f32 copy)
                    nc.vector.scalar_tensor_tensor(
                        d_sb[64:128, cs], h_cur[64:128, cs], 1.0, n_sb[64:128, cs],
                        MULT, mybir.AluOpType.subtract)
                    nc.vector.scalar_tensor_tensor(
                        d_sb[64:128, cs], d_sb[64:128, cs], 1.0, rz_sb[64:128, cs],
                        MULT, MULT)
                    nc.vector.scalar_tensor_tensor(
                        h_bf[64:128, cs], d_sb[64:128, cs], 1.0, n_sb[64:128, cs],
                        MULT, ADD)
                    nc.vector.scalar_tensor_tensor(
                        h_new[64:128, cs], d_sb[64:128, cs], 1.0, n_sb[64:128, cs],
                        MULT, ADD)
                # store hiddens[:, t] = h' (off-chain)
                nc.scalar.dma_start(out=hiddens_d[:, t, :], in_=h_new[64:128, :])
                # batched x-bar transposes -> hT
                hT = tb.tile([128, KH, BPC], BF16, tag="hT")
                for c in range(2):
                    eng = nc.sync if c % 2 == 0 else nc.scalar
                    eng.dma_start(
                        out=hT[:, 4 * c:4 * (c + 1), :],
                        in_=h_bf[64 + 32 * c:96 + 32 * c, :],
                        transpose=True)
                h_cur = h_new

                # ---- logits -> codes for next step (skip on last)
                if t < t_steps - 1:
                    logits_ps = ps.tile([128, 512], F32, tag="logits")
                    _emit_logits(nc, ones, bout, w_out, hT, logits_ps)
                    codesT = _emit_codes_epilogue(nc, (sb, sb1, tb), logits_ps,
                                                  samples_d, t + 1)

    nc.finalize()
    _BUILD_CACHE[t_steps] = nc
    return nc


def _pack_inputs(noise, W_ih, b_ih, W_hh, b_hh, W_out, b_out, t_steps=T):
    bf = ml_dtypes.bfloat16
    w_ih = np.ascontiguousarray(
        W_ih.T.astype(bf).reshape(KC, 128, G3).transpose(1, 0, 2))
    w_hh = np.ascontiguousarray(
        W_hh.T.astype(bf).reshape(KH, 128, G3).transpose(1, 0, 2))
    w_out = np.ascontiguousarray(
        W_out.T.astype(bf).reshape(KH, 128, C).transpose(1, 0, 2))
    brz = (b_ih + b_hh)[:2048].reshape(1, 2048).astype(bf)
    bn = np.concatenate([b_ih[2048:], b_hh[2048:]]).reshape(1, 2048).astype(bf)
    bout = b_out.reshape(1, 2048).astype(bf)

    shared = {"w_ih": w_ih, "w_hh": w_hh, "w_out": w_out,
              "bias_rz": brz, "bias_n": bn, "bias_out": bout}
    in_maps = []
    for i in range(NCORES):
        noiseT = np.ascontiguousarray(
            noise[BPC * i:BPC * (i + 1)].T.astype(bf)
            .reshape(KH, 128, BPC).transpose(1, 0, 2))
        in_maps.append({**shared, "noiseT": noiseT})
    return in_maps


def _run(noise, W_ih, b_ih, W_hh, b_hh, W_out, b_out, t_steps=T, **spmd_kwargs):
    nc = _build(t_steps)
    in_maps = _pack_inputs(noise, W_ih, b_ih, W_hh, b_hh, W_out, b_out, t_steps)
    res = run_bass_kernel_spmd(nc, in_maps, list(range(NCORES)), **spmd_kwargs)

    def unpack(name, nch, width):
        per_core = []
        for i in range(NCORES):
            p = res.results[i][name]  # [32*nch, t, 512] packed
            t_n = p.shape[1]
            per_core.append(
                p.reshape(nch, BPC, t_n, 512).transpose(1, 2, 0, 3)
                .reshape(BPC, t_n, width))
        return np.concatenate(per_core, 0)

    samples = unpack("samples", 4, C)
    hiddens = unpack("hiddens", 2, H)
    return (samples, hiddens), res


def kernel(noise, W_ih, b_ih, W_hh, b_hh, W_out, b_out, max_len):
    assert int(max_len) == T, f"kernel hardcodes T={T}, got {max_len}"
    noise = np.asarray(noise, dtype=np.float32)
    (samples, hiddens), _ = _run(
        noise, np.asarray(W_ih, np.float32), np.asarray(b_ih, np.float32),
        np.asarray(W_hh, np.float32), np.asarray(b_hh, np.float32),
        np.asarray(W_out, np.float32), np.asarray(b_out, np.float32))
    return samples, hiddens
